# revision 2
# baseline (speedup 1.0000x reference)
"""Trainium2 Bass kernel for the 32-iteration 3x3 survival automaton.

Problem: x is a 4096x4096 binary fp32 grid. 32 iterations of:
    keep cell iff its 8-neighbor live count > 3  (zero 'SAME' padding)
Output: scalar sum(x) - sum(y_final).

Strategy (8 NeuronCores, SPMD, zero inter-core communication):
  - Row-shard: core c owns rows [512c, 512c+512) and loads them plus a
    32-row halo per side; the halo is consumed one row per iteration, so
    after 32 iterations the owned rows are exact with no core-to-core
    traffic. One guard row/col of zeros emulates the 'SAME' zero padding
    (dead cells stay dead, so guards self-maintain).
  - Per-core slab: 578 rows x 4098 cols bf16, five 128-partition row tiles
    (stride 120, 8-row overlap; seam rows refreshed by DMAs every KSH
    iterations).
  - Work is balanced across ALL FOUR compute engines per iteration:
      TensorE: vertical 3-tap conv as tridiagonal matmuls. Columns are
        processed in 2048-wide pairs (4 PSUM banks). "A" pairs use two
        accumulated streams, Tri@B + (Tri+16I)@y, so one ScalarE sigmoid
        both thresholds and keeps y binary (s = N8 + 17y vs 20.5). "S"
        pairs use a single stream Tri@Hy (Hy = l+c+r) -- their threshold
        is a fused VectorE scalar_tensor_tensor (s > 4.5)*y, saving the
        second PE stream.
      ScalarE: sigmoid thresholds for A pairs (saturates to exact 1.0 /
        ~1e-26), batched 2048 wide, 8 pairs/iter.
      VectorE: B = l+r adds for tiles 0-1 (+ half of 4), Hy = B+y adds
        for S pairs, and the fused STT thresholds for S pairs.
      GpSimdE: B = l+r adds for tiles 2-3 (+ half of 4) -- otherwise idle.
  - TensorE stationaries: tri (all streams) and m16 = tri+16I (A second
    stream). Group emission order alternates per tile so dedup merges
    back-to-back reloads of the same stationary.
  - Final reduction: accum_out on the last iteration's thresholds gives
    per-partition row sums per pair; masked ones-vector matmuls reduce
    to one scalar per core. Host sums 8 partials, subtracts from sum(x).
"""

import sys

if '/opt/trn_rl_repo' not in sys.path:
    sys.path.insert(0, '/opt/trn_rl_repo')

from contextlib import ExitStack, contextmanager

import ml_dtypes
import numpy as np

import concourse.bass as bass
import concourse.tile as tile
from concourse import bacc, mybir
from concourse.bass_utils import run_bass_kernel_spmd

# ---------------------------------------------------------------- geometry
H = W = 4096
NCORES = 8
OWN = H // NCORES            # 512 rows owned per core
HALO = 32                    # rows of redundant compute per side
SLAB_R = OWN + 2 * HALO + 2  # 578 (incl. 1 guard row each side)
SLAB_C = W + 2               # 4098 (incl. 1 guard col each side)
NT = 5                       # SBUF row-tiles per slab
KSH = 4                      # seam shrink depth: refresh every KSH iters
STRIDE = 128 - 2 * KSH       # 120 (8-row overlap between tiles)
OFF = [t * STRIDE for t in range(NT)]              # 0,120,240,360,480
RT = [min(128, SLAB_R - o) for o in OFF]           # 128,128,128,128,98
PAIRW = 2048                 # threshold granularity: 4 PSUM banks
NPAIR = W // PAIRW           # 2 psum pairs per row-tile
MMW = 512                    # matmul output free size (1 PSUM bank)
MPP = PAIRW // MMW           # matmuls per pair per stationary (4)

# Per-tile pair schemes: 'A' = two-stream fold + ScalarE sigmoid;
# 'S' = single-stream Tri@Hy + fused VectorE (s>4.5)*y threshold.
SCHEMES = [('A', 'A'), ('A', 'A'), ('A', 'A'), ('A', 'S'), ('A', 'S')]
# B-pass (b = l + r) column ranges handled by GpSimd, per tile (rest on
# VectorE). Ranges are in b-tile columns [0, W).
GPS_B = {2: (0, W), 3: (0, W), 4: (0, PAIRW)}

F32 = mybir.dt.float32
BF16 = mybir.dt.bfloat16


@contextmanager
def _no_ldweights():
    """Emit InstMatmult with ldweights=False: reuse the PE array's currently
    loaded stationary instead of reloading per matmul."""
    orig = mybir.InstMatmult

    def mk(*a, **kw):
        kw['ldweights'] = False
        return orig(*a, **kw)

    mybir.InstMatmult = mk
    try:
        yield
    finally:
        mybir.InstMatmult = orig


def _ldw_sig(inst):
    """Signature of the stationary operand an InstLdweights loads."""
    ap = inst.ins[0]
    return (getattr(ap, 'memref', None), getattr(ap, 'offset', None),
            str(getattr(ap, 'ap', None)), str(inst.tile_position),
            str(inst.tile_size), str(getattr(inst, 'perf_mode', None)),
            str(getattr(inst, 'is_transpose', None)))


def _dedup_ldweights(nc):
    """Remove InstLdweights that reload the stationary already in the PE
    array (same weights AP, only non-loading Matmults in between). Waits on
    a removed load are pushed onto the next PE instruction; loads carrying
    semaphore updates are kept."""
    removed = 0
    for f in nc.m.functions:
        for blk in f.blocks:
            cur = None
            out = []
            pending_waits = []
            for inst in blk.instructions:
                if isinstance(inst, mybir.InstLdweights):
                    sig = _ldw_sig(inst)
                    si = inst.sync_info
                    has_upd = si is not None and len(si.on_update) > 0
                    if sig == cur and not has_upd:
                        if si is not None and len(si.on_wait) > 0:
                            pending_waits.extend(si.on_wait)
                        removed += 1
                        continue
                    cur = sig
                elif isinstance(inst, mybir.InstMatmult):
                    if inst.is_transpose or getattr(inst, 'ldweights', None) is not False:
                        cur = None
                elif type(inst).__name__ == 'InstMatmultMx':
                    cur = None
                if pending_waits and isinstance(
                        inst, (mybir.InstLdweights, mybir.InstMatmult)):
                    si = inst.sync_info
                    if si is None:
                        inst.sync_info = mybir.SyncInfo(
                            on_wait=list(pending_waits), on_update=[])
                    else:
                        si.on_wait = list(si.on_wait) + pending_waits
                    pending_waits = []
                out.append(inst)
            assert not pending_waits
            if len(out) != len(blk.instructions):
                blk.instructions[:] = out
    return removed


def _build(iters: int):
    nc = bacc.Bacc("TRN2", target_bir_lowering=False, debug=False)
    x_d = nc.dram_tensor("x", [SLAB_R, SLAB_C], BF16, kind="ExternalInput").ap()
    tri_d = nc.dram_tensor("tri", [128, 128], BF16, kind="ExternalInput").ap()
    m16_d = nc.dram_tensor("m16", [128, 128], BF16, kind="ExternalInput").ap()
    rmask_d = nc.dram_tensor("rmask", [NT, 128], F32, kind="ExternalInput").ap()
    out_d = nc.dram_tensor("ysum", [1, 1], F32, kind="ExternalOutput").ap()

    add = mybir.AluOpType.add

    with tile.TileContext(nc) as tc, ExitStack() as ctx:
        const_pool = ctx.enter_context(tc.tile_pool(name="const", bufs=1))
        ypool = ctx.enter_context(tc.tile_pool(name="y", bufs=1))
        bpool = ctx.enter_context(tc.tile_pool(name="b", bufs=1))

        tri_sb = const_pool.tile([128, 128], BF16, tag="tri")
        nc.sync.dma_start(tri_sb[:], tri_d[:])
        m16_sb = const_pool.tile([128, 128], BF16, tag="m16")
        nc.sync.dma_start(m16_sb[:], m16_d[:])
        rmask_sb = []
        for t in range(NT):
            rm = const_pool.tile([128, 1], F32, tag=f"rmask{t}", name=f"rmask{t}")
            nc.sync.dma_start(rm[:], rmask_d[t:t + 1, :])
            rmask_sb.append(rm)
        bias_sb = const_pool.tile([128, 1], F32, tag="biasc", name="biasc")
        nc.gpsimd.memset(bias_sb[:], -2460.0)

        y_sb = [ypool.tile([RT[t], SLAB_C], BF16, tag=f"y{t}", name=f"y{t}")
                for t in range(NT)]
        b_sb = [bpool.tile([RT[t], W], BF16, tag=f"b{t}", name=f"b{t}")
                for t in range(NT)]
        # Hy scratch for S pairs (b + center), one per tile that has any
        hy_sb = {}
        for t in range(NT):
            for p, sch in enumerate(SCHEMES[t]):
                if sch == 'S':
                    hy_sb[(t, p)] = bpool.tile(
                        [RT[t], PAIRW], BF16, tag=f"hy{t}_{p}", name=f"hy{t}_{p}")

        # load (host already converted to bf16)
        for t in range(NT):
            nc.sync.dma_start(y_sb[t][:], x_d[OFF[t]:OFF[t] + RT[t], :])

        def emit_adds(t):
            # b = l + r over all cols, split DVE / GpSimd by GPS_B ranges
            g = GPS_B.get(t)
            r = RT[t]
            if g is None:
                nc.vector.tensor_tensor(
                    b_sb[t][:], y_sb[t][:, 0:W], y_sb[t][:, 2:W + 2], op=add)
            else:
                g0, g1 = g
                nc.gpsimd.tensor_tensor(
                    b_sb[t][0:r, g0:g1], y_sb[t][:, g0:g1],
                    y_sb[t][:, g0 + 2:g1 + 2], op=add)
                if g0 > 0:
                    nc.vector.tensor_tensor(
                        b_sb[t][0:r, 0:g0], y_sb[t][:, 0:g0],
                        y_sb[t][:, 2:g0 + 2], op=add)
                if g1 < W:
                    nc.vector.tensor_tensor(
                        b_sb[t][0:r, g1:W], y_sb[t][:, g1:W],
                        y_sb[t][:, g1 + 2:W + 2], op=add)
            # Hy = b + center for S pairs
            for p, sch in enumerate(SCHEMES[t]):
                if sch == 'S':
                    c0 = p * PAIRW
                    nc.vector.tensor_tensor(
                        hy_sb[(t, p)][:], b_sb[t][0:r, c0:c0 + PAIRW],
                        y_sb[t][:, 1 + c0:1 + c0 + PAIRW], op=add)

        def emit_seam(t):
            # refresh the 2*KSH-row overlap between tiles t and t+1 (each
            # tile's outer KSH rows go stale over KSH iterations)
            nc.sync.dma_start(y_sb[t][128 - KSH:128, :],
                              y_sb[t + 1][KSH:2 * KSH, :])
            nc.sync.dma_start(y_sb[t + 1][0:KSH, :],
                              y_sb[t][STRIDE:STRIDE + KSH, :])

        acc_sb = [[const_pool.tile([128, 1], F32, tag=f"acc{t}_{p}",
                                   name=f"acc{t}_{p}") for p in range(NPAIR)]
                  for t in range(NT)]

        def mm(first, *args, **kw):
            if first:
                nc.tensor.matmul(*args, **kw)
            else:
                with _no_ldweights():
                    nc.tensor.matmul(*args, **kw)

        def emit_mms_thresholds(psum_pool, it, t, accum=False):
            r = RT[t]
            psums = [psum_pool.tile([r, PAIRW], F32, tag="ps",
                                    name=f"ps_{it}_{t}_{p}")
                     for p in range(NPAIR)]
            a_pairs = [p for p in range(NPAIR) if SCHEMES[t][p] == 'A']
            s_pairs = [p for p in range(NPAIR) if SCHEMES[t][p] == 'S']

            def tri_group(first, a_start):
                # tri stream: A pairs read b, S pairs read hy
                for p in a_pairs:
                    for h in range(MPP):
                        c0 = p * PAIRW + h * MMW
                        mm(first, psums[p][:, h * MMW:(h + 1) * MMW],
                           tri_sb[0:r, 0:r], b_sb[t][0:r, c0:c0 + MMW],
                           start=a_start, stop=not a_start)
                        first = False
                for p in s_pairs:
                    for h in range(MPP):
                        mm(first, psums[p][:, h * MMW:(h + 1) * MMW],
                           tri_sb[0:r, 0:r],
                           hy_sb[(t, p)][0:r, h * MMW:(h + 1) * MMW],
                           start=True, stop=True)
                        first = False

            def m16_group(first, a_start):
                for p in a_pairs:
                    for h in range(MPP):
                        c0 = p * PAIRW + h * MMW
                        mm(first, psums[p][:, h * MMW:(h + 1) * MMW],
                           m16_sb[0:r, 0:r],
                           y_sb[t][:, 1 + c0:1 + c0 + MMW],
                           start=a_start, stop=not a_start)
                        first = False

            # alternate group order by tile parity so adjacent tiles end /
            # begin with the same stationary (dedup merges the reload)
            if t % 2 == 0:
                tri_group(True, True)
                m16_group(True, False)
            else:
                m16_group(True, True)
                tri_group(True, False)

            for p in range(NPAIR):
                aout = acc_sb[t][p][0:r, 0:1] if accum else None
                if SCHEMES[t][p] == 'A':
                    dst = y_sb[t][:, 1 + p * PAIRW:1 + (p + 1) * PAIRW]
                    nc.scalar.activation(
                        dst, psums[p][:],
                        mybir.ActivationFunctionType.Sigmoid,
                        bias=bias_sb[0:r, 0:1], scale=120.0,
                        accum_out=aout)
                else:
                    ysl = y_sb[t][:, 1 + p * PAIRW:1 + (p + 1) * PAIRW]
                    nc.vector.scalar_tensor_tensor(
                        ysl, psums[p][:], 4.5, ysl,
                        op0=mybir.AluOpType.is_gt,
                        op1=mybir.AluOpType.mult,
                        accum_out=aout)

        # Software-pipelined wavefront with seam shrinkage: tiles overlap by
        # 2*KSH rows, so seams need refreshing only every KSH-th iteration.
        # On non-refresh boundaries a tile's next-iteration adds depend only
        # on its own thresholds and are emitted right after it -- TensorE
        # rolls across the iteration boundary with no bubble. On refresh
        # boundaries, seams are refreshed as soon as both neighbor tiles are
        # thresholded.
        with tc.tile_pool(name="ps", bufs=2, space="PSUM") as psum_pool:
            for t in range(NT):
                emit_adds(t)
            for it in range(iters):
                last = it == iters - 1
                refresh = (it % KSH == KSH - 1) and not last
                for t in range(NT):
                    emit_mms_thresholds(psum_pool, it, t, accum=last)
                    if last:
                        continue
                    if refresh:
                        if t >= 1:
                            emit_seam(t - 1)
                        if t >= 2:
                            emit_adds(t - 2)
                    else:
                        emit_adds(t)
                if not last and refresh:
                    emit_adds(NT - 2)
                    emit_adds(NT - 1)

        # masked dot of the per-row accumulators from the last iteration's
        # thresholds: ysum = sum_t rmask[t] . (row sums of tile t)
        with tc.tile_pool(name="sps", bufs=1, space="PSUM") as spsum_pool:
            sps = spsum_pool.tile([1, 1], F32, tag="sum", name="sps")
            n_mm = NT * NPAIR
            k = 0
            for t in range(NT):
                for p in range(NPAIR):
                    nc.tensor.matmul(
                        sps[:], rmask_sb[t][0:RT[t], 0:1],
                        acc_sb[t][p][0:RT[t], 0:1],
                        start=(k == 0), stop=(k == n_mm - 1))
                    k += 1
            ssb = const_pool.tile([1, 1], F32, tag="ssum", name="ssb")
            nc.vector.tensor_copy(ssb[:], sps[:])
            nc.sync.dma_start(out_d[:], ssb[:])

    _dedup_ldweights(nc)
    # After dedup, the "most recent ldweights" a matmul's extra waits would
    # be moved to can sit many matmuls earlier in the PE stream — waiting
    # there can deadlock against producers scheduled in between. Skip the
    # pass; generate_event_semaphores enforces the 1-wait constraint by
    # splitting waits into standalone event-sem instructions in place.
    nc.move_matmul_waits_to_ldweights = lambda: None
    nc.compile()
    return nc


def _consts():
    i = np.arange(128)
    tri = (np.abs(i[:, None] - i[None, :]) <= 1).astype(np.float32)
    m16 = tri + 16.0 * np.eye(128, dtype=np.float32)
    # valid-row masks for the final sum: slab rows [33, 545) are the owned
    # 512 rows; each row is summed from the tile where it is seam-valid
    # (interior partitions after the last iteration).
    rmask = np.zeros((NT, 128), np.float32)
    bounds = [(33, 124), (4, 124), (4, 124), (4, 124), (4, 65)]
    for t, (a, b) in enumerate(bounds):
        rmask[t, a:b] = 1.0
    assert sum(b - a for a, b in bounds) == OWN
    bf = ml_dtypes.bfloat16
    return tri.astype(bf), m16.astype(bf), rmask


def _slabs(x: np.ndarray):
    g = np.zeros((H + 2 * HALO + 2, SLAB_C), ml_dtypes.bfloat16)
    g[HALO + 1:HALO + 1 + H, 1:1 + W] = x  # 0/1 values: exact in bf16
    return [np.ascontiguousarray(g[c * OWN:c * OWN + SLAB_R])
            for c in range(NCORES)]


_CACHE = {}


def _get_nc(iters: int):
    if iters not in _CACHE:
        _CACHE[iters] = _build(iters)
    return _CACHE[iters]


def kernel(x: np.ndarray, convs) -> np.ndarray:
    iters = int(convs)
    x = np.asarray(x, np.float32)
    assert x.shape == (H, W)
    nc = _get_nc(iters)
    tri, m16, rmask = _consts()
    in_maps = [{"x": s, "tri": tri, "m16": m16, "rmask": rmask}
               for s in _slabs(x)]
    res = run_bass_kernel_spmd(nc, in_maps, core_ids=list(range(NCORES)))
    y_sum = sum(float(res.results[c]["ysum"][0, 0]) for c in range(NCORES))
    x_sum = float(x.astype(np.float64).sum())
    return np.float32(x_sum - y_sum)


if __name__ == "__main__":
    rng = np.random.default_rng(0)
    x = np.round(rng.random((H, W))).astype(np.float32)
    got = kernel(x, 32)
    from scipy import signal
    K = np.array([[1, 1, 1], [1, 0, 1], [1, 1, 1]], np.float32)
    y = x.copy()
    for _ in range(32):
        s = signal.convolve2d(y, K, mode='same')
        y = np.where(s > 3.0, y, 0).astype(np.float32)
    want = x.sum(dtype=np.float64) - y.sum(dtype=np.float64)
    print(f"got {got}, want {want}, rel {abs(got - want) / abs(want):.3e}")


# revision 6
# speedup vs baseline: 1.1986x; 1.1986x over previous
"""Trainium2 Bass kernel for the 32-iteration 3x3 survival automaton.

Problem: x is a 4096x4096 binary fp32 grid. 32 iterations of:
    keep cell iff its 8-neighbor live count > 3  (zero 'SAME' padding)
Output: scalar sum(x) - sum(y_final).

Strategy (8 NeuronCores, SPMD, zero inter-core communication):
  - Row-shard: core c owns rows [512c, 512c+512) and loads them plus a
    32-row halo per side; the halo is consumed one row per iteration, so
    after 32 iterations the owned rows are exact with no core-to-core
    traffic. One guard row/col of zeros emulates the 'SAME' zero padding
    (dead cells stay dead, so guards self-maintain).
  - Per-core slab: 578 rows x 4098 cols bf16, five 128-partition row tiles
    (stride 120, 8-row overlap; seam rows refreshed by DMAs every KSH
    iterations).
  - Work is balanced across ALL FOUR compute engines per iteration:
      TensorE: vertical 3-tap conv as tridiagonal matmuls. Columns are
        processed in 2048-wide pairs (4 PSUM banks). "A" pairs use two
        accumulated streams, Tri@B + (Tri+16I)@y, so one ScalarE sigmoid
        both thresholds and keeps y binary (s = N8 + 17y vs 20.5). "S"
        pairs use a single stream Tri@Hy (Hy = l+c+r) -- their threshold
        is a fused VectorE scalar_tensor_tensor (s > 4.5)*y, saving the
        second PE stream.
      ScalarE: sigmoid thresholds for A pairs (saturates to exact 1.0 /
        ~1e-26), batched 2048 wide, 8 pairs/iter.
      VectorE: B = l+r adds for tiles 0-1 (+ half of 4), Hy = B+y adds
        for S pairs, and the fused STT thresholds for S pairs.
      GpSimdE: B = l+r adds for tiles 2-3 (+ half of 4) -- otherwise idle.
  - TensorE stationaries: tri (all streams) and m16 = tri+16I (A second
    stream). Group emission order alternates per tile so dedup merges
    back-to-back reloads of the same stationary.
  - Final reduction: accum_out on the last iteration's thresholds gives
    per-partition row sums per pair; masked ones-vector matmuls reduce
    to one scalar per core. Host sums 8 partials, subtracts from sum(x).
"""

import sys

if '/opt/trn_rl_repo' not in sys.path:
    sys.path.insert(0, '/opt/trn_rl_repo')

from contextlib import ExitStack, contextmanager

import ml_dtypes
import numpy as np

import concourse.bass as bass
import concourse.tile as tile
from concourse import bacc, mybir
from concourse.bass_utils import run_bass_kernel_spmd

# ---------------------------------------------------------------- geometry
H = W = 4096
NCORES = 8
OWN = H // NCORES            # 512 rows owned per core
HALO = 32                    # rows of redundant compute per side
SLAB_R = OWN + 2 * HALO + 2  # 578 (incl. 1 guard row each side)
SLAB_C = W + 2               # 4098 (incl. 1 guard col each side)
NT = 5                       # SBUF row-tiles per slab
KSH = 4                      # seam shrink depth: refresh every KSH iters
STRIDE = 128 - 2 * KSH       # 120 (8-row overlap between tiles)
OFF = [t * STRIDE for t in range(NT)]              # 0,120,240,360,480
RT = [min(128, SLAB_R - o) for o in OFF]           # 128,128,128,128,98
PAIRW = 2048                 # threshold granularity: 4 PSUM banks
NPAIR = W // PAIRW           # 2 psum pairs per row-tile
MMW = 512                    # matmul output free size (1 PSUM bank)
MPP = PAIRW // MMW           # matmuls per pair per stationary (4)

# Per-tile count of trailing 512-col chunks (of 8) using the 'S' scheme:
# single-stream Tri@Hy + fused VectorE (s>4.5)*y threshold. The leading
# chunks use 'A': two-stream fold + ScalarE sigmoid. Tuned so PE, ACT and
# DVE land at the same per-iteration busy time (~16.7us).
S_CHUNKS = [0, 0, 0, 4, 3]

F32 = mybir.dt.float32
BF16 = mybir.dt.bfloat16


@contextmanager
def _no_ldweights():
    """Emit InstMatmult with ldweights=False: reuse the PE array's currently
    loaded stationary instead of reloading per matmul."""
    orig = mybir.InstMatmult

    def mk(*a, **kw):
        kw['ldweights'] = False
        return orig(*a, **kw)

    mybir.InstMatmult = mk
    try:
        yield
    finally:
        mybir.InstMatmult = orig


def _ldw_sig(inst):
    """Signature of the stationary operand an InstLdweights loads."""
    ap = inst.ins[0]
    return (getattr(ap, 'memref', None), getattr(ap, 'offset', None),
            str(getattr(ap, 'ap', None)), str(inst.tile_position),
            str(inst.tile_size), str(getattr(inst, 'perf_mode', None)),
            str(getattr(inst, 'is_transpose', None)))


def _dedup_ldweights(nc):
    """Remove InstLdweights that reload the stationary already in the PE
    array (same weights AP, only non-loading Matmults in between). Waits on
    a removed load are pushed onto the next PE instruction; loads carrying
    semaphore updates are kept."""
    removed = 0
    for f in nc.m.functions:
        for blk in f.blocks:
            cur = None
            out = []
            pending_waits = []
            for inst in blk.instructions:
                if isinstance(inst, mybir.InstLdweights):
                    sig = _ldw_sig(inst)
                    si = inst.sync_info
                    has_upd = si is not None and len(si.on_update) > 0
                    if sig == cur and not has_upd:
                        if si is not None and len(si.on_wait) > 0:
                            pending_waits.extend(si.on_wait)
                        removed += 1
                        continue
                    cur = sig
                elif isinstance(inst, mybir.InstMatmult):
                    if inst.is_transpose or getattr(inst, 'ldweights', None) is not False:
                        cur = None
                elif type(inst).__name__ == 'InstMatmultMx':
                    cur = None
                if pending_waits and isinstance(
                        inst, (mybir.InstLdweights, mybir.InstMatmult)):
                    si = inst.sync_info
                    if si is None:
                        inst.sync_info = mybir.SyncInfo(
                            on_wait=list(pending_waits), on_update=[])
                    else:
                        si.on_wait = list(si.on_wait) + pending_waits
                    pending_waits = []
                out.append(inst)
            assert not pending_waits
            if len(out) != len(blk.instructions):
                blk.instructions[:] = out
    return removed


def _build(iters: int):
    nc = bacc.Bacc("TRN2", target_bir_lowering=False, debug=False)
    x_d = nc.dram_tensor("x", [SLAB_R, SLAB_C], BF16, kind="ExternalInput").ap()
    tri_d = nc.dram_tensor("tri", [128, 128], BF16, kind="ExternalInput").ap()
    m16_d = nc.dram_tensor("m16", [128, 128], BF16, kind="ExternalInput").ap()
    rmask_d = nc.dram_tensor("rmask", [NT, 128], F32, kind="ExternalInput").ap()
    out_d = nc.dram_tensor("ysum", [1, 1], F32, kind="ExternalOutput").ap()

    add = mybir.AluOpType.add

    with tile.TileContext(nc) as tc, ExitStack() as ctx:
        const_pool = ctx.enter_context(tc.tile_pool(name="const", bufs=1))
        ypool = ctx.enter_context(tc.tile_pool(name="y", bufs=1))
        bpool = ctx.enter_context(tc.tile_pool(name="b", bufs=1))

        tri_sb = const_pool.tile([128, 128], BF16, tag="tri")
        nc.sync.dma_start(tri_sb[:], tri_d[:])
        m16_sb = const_pool.tile([128, 128], BF16, tag="m16")
        nc.sync.dma_start(m16_sb[:], m16_d[:])
        rmask_sb = []
        for t in range(NT):
            rm = const_pool.tile([128, 1], F32, tag=f"rmask{t}", name=f"rmask{t}")
            nc.sync.dma_start(rm[:], rmask_d[t:t + 1, :])
            rmask_sb.append(rm)
        bias_sb = const_pool.tile([128, 1], F32, tag="biasc", name="biasc")
        nc.gpsimd.memset(bias_sb[:], -2460.0)

        y_sb = [ypool.tile([RT[t], SLAB_C], BF16, tag=f"y{t}", name=f"y{t}")
                for t in range(NT)]
        b_sb = [bpool.tile([RT[t], W], BF16, tag=f"b{t}", name=f"b{t}")
                for t in range(NT)]
        # Hy scratch for S chunks (b + center); S chunks are the trailing
        # S_CHUNKS[t]*512 columns of tile t
        SW = [S_CHUNKS[t] * MMW for t in range(NT)]      # S width per tile
        SC0 = [W - SW[t] for t in range(NT)]             # S col start
        hy_sb = {t: bpool.tile([RT[t], SW[t]], BF16, tag=f"hy{t}",
                               name=f"hy{t}")
                 for t in range(NT) if SW[t] > 0}

        # load (host already converted to bf16)
        for t in range(NT):
            nc.sync.dma_start(y_sb[t][:], x_d[OFF[t]:OFF[t] + RT[t], :])

        def emit_adds(t):
            r = RT[t]
            # b = l + r over all cols
            nc.vector.tensor_tensor(
                b_sb[t][:], y_sb[t][:, 0:W], y_sb[t][:, 2:W + 2], op=add)
            # Hy = b + center for the S range
            if SW[t] > 0:
                c0 = SC0[t]
                nc.vector.tensor_tensor(
                    hy_sb[t][:], b_sb[t][0:r, c0:W],
                    y_sb[t][:, 1 + c0:1 + W], op=add)

        def emit_seam(t):
            # refresh the 2*KSH-row overlap between tiles t and t+1 (each
            # tile's outer KSH rows go stale over KSH iterations)
            nc.sync.dma_start(y_sb[t][128 - KSH:128, :],
                              y_sb[t + 1][KSH:2 * KSH, :])
            nc.sync.dma_start(y_sb[t + 1][0:KSH, :],
                              y_sb[t][STRIDE:STRIDE + KSH, :])

        acc_list = []  # (tile, acc_tile) pairs written on the last iteration

        def mm(first, *args, **kw):
            if first:
                nc.tensor.matmul(*args, **kw)
            else:
                with _no_ldweights():
                    nc.tensor.matmul(*args, **kw)

        def emit_mms_thresholds(psum_pool, it, t, accum=False):
            r = RT[t]
            psums = [psum_pool.tile([r, PAIRW], F32, tag="ps",
                                    name=f"ps_{it}_{t}_{p}")
                     for p in range(NPAIR)]
            n_chunks = W // MMW
            a_end = n_chunks - S_CHUNKS[t]   # chunks [0, a_end) are 'A'

            def tri_group(first, a_start):
                # tri stream: A chunks read b, S chunks read hy
                for ch in range(n_chunks):
                    p, h = divmod(ch, MPP)
                    dst = psums[p][:, h * MMW:(h + 1) * MMW]
                    c0 = ch * MMW
                    if ch < a_end:
                        mm(first, dst, tri_sb[0:r, 0:r],
                           b_sb[t][0:r, c0:c0 + MMW],
                           start=a_start, stop=not a_start)
                    else:
                        mm(first, dst, tri_sb[0:r, 0:r],
                           hy_sb[t][0:r, c0 - SC0[t]:c0 - SC0[t] + MMW],
                           start=True, stop=True)
                    first = False

            def m16_group(first, a_start):
                for ch in range(a_end):
                    p, h = divmod(ch, MPP)
                    c0 = ch * MMW
                    mm(first, psums[p][:, h * MMW:(h + 1) * MMW],
                       m16_sb[0:r, 0:r],
                       y_sb[t][:, 1 + c0:1 + c0 + MMW],
                       start=a_start, stop=not a_start)
                    first = False

            # alternate group order by tile parity so adjacent tiles end /
            # begin with the same stationary (dedup merges the reload)
            if t % 2 == 0:
                tri_group(True, True)
                m16_group(True, False)
            else:
                m16_group(True, True)
                tri_group(True, False)

            def acc_for(kind):
                if not accum:
                    return None
                a = const_pool.tile([128, 1], F32, tag=f"acc{t}_{kind}",
                                    name=f"acc{t}_{kind}")
                acc_list.append((t, a))
                return a[0:r, 0:1]

            for p in range(NPAIR):
                lo, hi = p * MPP, (p + 1) * MPP       # chunk range of pair
                a_hi = min(hi, a_end)
                if a_hi > lo:                          # A sub-range: sigmoid
                    c0, c1 = lo * MMW, a_hi * MMW
                    nc.scalar.activation(
                        y_sb[t][:, 1 + c0:1 + c1],
                        psums[p][:, c0 - lo * MMW:c1 - lo * MMW],
                        mybir.ActivationFunctionType.Sigmoid,
                        bias=bias_sb[0:r, 0:1], scale=120.0,
                        accum_out=acc_for(f"{p}a"))
                if hi > max(lo, a_end):                # S sub-range: fused STT
                    c0, c1 = max(lo, a_end) * MMW, hi * MMW
                    ysl = y_sb[t][:, 1 + c0:1 + c1]
                    nc.vector.scalar_tensor_tensor(
                        ysl, psums[p][:, c0 - lo * MMW:c1 - lo * MMW],
                        4.5, ysl,
                        op0=mybir.AluOpType.is_gt,
                        op1=mybir.AluOpType.mult,
                        accum_out=acc_for(f"{p}s"))

        # Software-pipelined wavefront with seam shrinkage: tiles overlap by
        # 2*KSH rows, so seams need refreshing only every KSH-th iteration.
        # On non-refresh boundaries a tile's next-iteration adds depend only
        # on its own thresholds and are emitted right after it -- TensorE
        # rolls across the iteration boundary with no bubble. On refresh
        # boundaries, seams are refreshed as soon as both neighbor tiles are
        # thresholded.
        with tc.tile_pool(name="ps", bufs=2, space="PSUM") as psum_pool:
            for t in range(NT):
                emit_adds(t)
            for it in range(iters):
                last = it == iters - 1
                refresh = (it % KSH == KSH - 1) and not last
                for t in range(NT):
                    emit_mms_thresholds(psum_pool, it, t, accum=last)
                    if last:
                        continue
                    if refresh:
                        if t >= 1:
                            emit_seam(t - 1)
                        if t >= 2:
                            emit_adds(t - 2)
                    else:
                        emit_adds(t)
                if not last and refresh:
                    emit_adds(NT - 2)
                    emit_adds(NT - 1)

        # masked dot of the per-row accumulators from the last iteration's
        # thresholds: ysum = sum_t rmask[t] . (row sums of tile t)
        with tc.tile_pool(name="sps", bufs=1, space="PSUM") as spsum_pool:
            sps = spsum_pool.tile([1, 1], F32, tag="sum", name="sps")
            n_mm = len(acc_list)
            for k, (t, a) in enumerate(acc_list):
                nc.tensor.matmul(
                    sps[:], rmask_sb[t][0:RT[t], 0:1],
                    a[0:RT[t], 0:1],
                    start=(k == 0), stop=(k == n_mm - 1))
            ssb = const_pool.tile([1, 1], F32, tag="ssum", name="ssb")
            nc.vector.tensor_copy(ssb[:], sps[:])
            nc.sync.dma_start(out_d[:], ssb[:])

    _dedup_ldweights(nc)
    # After dedup, the "most recent ldweights" a matmul's extra waits would
    # be moved to can sit many matmuls earlier in the PE stream — waiting
    # there can deadlock against producers scheduled in between. Skip the
    # pass; generate_event_semaphores enforces the 1-wait constraint by
    # splitting waits into standalone event-sem instructions in place.
    nc.move_matmul_waits_to_ldweights = lambda: None
    nc.compile()
    return nc


def _consts():
    i = np.arange(128)
    tri = (np.abs(i[:, None] - i[None, :]) <= 1).astype(np.float32)
    m16 = tri + 16.0 * np.eye(128, dtype=np.float32)
    # valid-row masks for the final sum: slab rows [33, 545) are the owned
    # 512 rows; each row is summed from the tile where it is seam-valid
    # (interior partitions after the last iteration).
    rmask = np.zeros((NT, 128), np.float32)
    bounds = [(33, 124), (4, 124), (4, 124), (4, 124), (4, 65)]
    for t, (a, b) in enumerate(bounds):
        rmask[t, a:b] = 1.0
    assert sum(b - a for a, b in bounds) == OWN
    bf = ml_dtypes.bfloat16
    return tri.astype(bf), m16.astype(bf), rmask


def _slabs(x: np.ndarray):
    g = np.zeros((H + 2 * HALO + 2, SLAB_C), ml_dtypes.bfloat16)
    g[HALO + 1:HALO + 1 + H, 1:1 + W] = x  # 0/1 values: exact in bf16
    return [np.ascontiguousarray(g[c * OWN:c * OWN + SLAB_R])
            for c in range(NCORES)]


_CACHE = {}


def _get_nc(iters: int):
    if iters not in _CACHE:
        _CACHE[iters] = _build(iters)
    return _CACHE[iters]


def kernel(x: np.ndarray, convs) -> np.ndarray:
    iters = int(convs)
    x = np.asarray(x, np.float32)
    assert x.shape == (H, W)
    nc = _get_nc(iters)
    tri, m16, rmask = _consts()
    in_maps = [{"x": s, "tri": tri, "m16": m16, "rmask": rmask}
               for s in _slabs(x)]
    res = run_bass_kernel_spmd(nc, in_maps, core_ids=list(range(NCORES)))
    y_sum = sum(float(res.results[c]["ysum"][0, 0]) for c in range(NCORES))
    x_sum = float(x.astype(np.float64).sum())
    return np.float32(x_sum - y_sum)


if __name__ == "__main__":
    rng = np.random.default_rng(0)
    x = np.round(rng.random((H, W))).astype(np.float32)
    got = kernel(x, 32)
    from scipy import signal
    K = np.array([[1, 1, 1], [1, 0, 1], [1, 1, 1]], np.float32)
    y = x.copy()
    for _ in range(32):
        s = signal.convolve2d(y, K, mode='same')
        y = np.where(s > 3.0, y, 0).astype(np.float32)
    want = x.sum(dtype=np.float64) - y.sum(dtype=np.float64)
    print(f"got {got}, want {want}, rel {abs(got - want) / abs(want):.3e}")


# revision 9
# speedup vs baseline: 1.2295x; 1.0258x over previous
"""Trainium2 Bass kernel for the 32-iteration 3x3 survival automaton.

Problem: x is a 4096x4096 binary fp32 grid. 32 iterations of:
    keep cell iff its 8-neighbor live count > 3  (zero 'SAME' padding)
Output: scalar sum(x) - sum(y_final).

Strategy (8 NeuronCores, SPMD, zero inter-core communication):
  - Row-shard: core c owns rows [512c, 512c+512) and loads them plus a
    32-row halo per side; the halo is consumed one row per iteration, so
    after 32 iterations the owned rows are exact with no core-to-core
    traffic. One guard row/col of zeros emulates the 'SAME' zero padding
    (dead cells stay dead, so guards self-maintain).
  - Per-core slab: 578 rows x 4098 cols bf16, five 128-partition row tiles
    (stride 120, 8-row overlap; seam rows refreshed by DMAs every KSH
    iterations).
  - Work is balanced across ALL FOUR compute engines per iteration:
      TensorE: vertical 3-tap conv as tridiagonal matmuls. Columns are
        processed in 2048-wide pairs (4 PSUM banks). "A" pairs use two
        accumulated streams, Tri@B + (Tri+16I)@y, so one ScalarE sigmoid
        both thresholds and keeps y binary (s = N8 + 17y vs 20.5). "S"
        pairs use a single stream Tri@Hy (Hy = l+c+r) -- their threshold
        is a fused VectorE scalar_tensor_tensor (s > 4.5)*y, saving the
        second PE stream.
      ScalarE: sigmoid thresholds for A pairs (saturates to exact 1.0 /
        ~1e-26), batched 2048 wide, 8 pairs/iter.
      VectorE: B = l+r adds for tiles 0-1 (+ half of 4), Hy = B+y adds
        for S pairs, and the fused STT thresholds for S pairs.
      GpSimdE: B = l+r adds for tiles 2-3 (+ half of 4) -- otherwise idle.
  - TensorE stationaries: tri (all streams) and m16 = tri+16I (A second
    stream). Group emission order alternates per tile so dedup merges
    back-to-back reloads of the same stationary.
  - Final reduction: accum_out on the last iteration's thresholds gives
    per-partition row sums per pair; masked ones-vector matmuls reduce
    to one scalar per core. Host sums 8 partials, subtracts from sum(x).
"""

import sys

if '/opt/trn_rl_repo' not in sys.path:
    sys.path.insert(0, '/opt/trn_rl_repo')

from contextlib import ExitStack, contextmanager

import ml_dtypes
import numpy as np

import concourse.bass as bass
import concourse.tile as tile
from concourse import bacc, mybir
from concourse.bass_utils import run_bass_kernel_spmd

# ---------------------------------------------------------------- geometry
H = W = 4096
NCORES = 8
OWN = H // NCORES            # 512 rows owned per core
HALO = 32                    # rows of redundant compute per side
SLAB_R = OWN + 2 * HALO + 2  # 578 (incl. 1 guard row each side)
SLAB_C = W + 2               # 4098 (incl. 1 guard col each side)
NT = 5                       # SBUF row-tiles per slab
KSH = 4                      # seam shrink depth: refresh every KSH iters
STRIDE = 128 - 2 * KSH       # 120 (8-row overlap between tiles)
OFF = [t * STRIDE for t in range(NT)]              # 0,120,240,360,480
RT = [min(128, SLAB_R - o) for o in OFF]           # 128,128,128,128,98
PAIRW = 2048                 # threshold granularity: 4 PSUM banks
NPAIR = W // PAIRW           # 2 psum pairs per row-tile
MMW = 512                    # matmul output free size (1 PSUM bank)
MPP = PAIRW // MMW           # matmuls per pair per stationary (4)

# Per-tile count of trailing 512-col chunks (of 8) using the 'S' scheme:
# single-stream Tri@Hy + fused VectorE (s>4.5)*y threshold. The leading
# chunks use 'A': two-stream fold + ScalarE sigmoid. Tuned so PE, ACT and
# DVE land at the same per-iteration busy time (~16.7us).
S_CHUNKS = [0, 0, 0, 4, 3]

F32 = mybir.dt.float32
BF16 = mybir.dt.bfloat16


@contextmanager
def _no_ldweights():
    """Emit InstMatmult with ldweights=False: reuse the PE array's currently
    loaded stationary instead of reloading per matmul."""
    orig = mybir.InstMatmult

    def mk(*a, **kw):
        kw['ldweights'] = False
        return orig(*a, **kw)

    mybir.InstMatmult = mk
    try:
        yield
    finally:
        mybir.InstMatmult = orig


def _ldw_sig(inst):
    """Signature of the stationary operand an InstLdweights loads."""
    ap = inst.ins[0]
    return (getattr(ap, 'memref', None), getattr(ap, 'offset', None),
            str(getattr(ap, 'ap', None)), str(inst.tile_position),
            str(inst.tile_size), str(getattr(inst, 'perf_mode', None)),
            str(getattr(inst, 'is_transpose', None)))


def _dedup_ldweights(nc):
    """Remove InstLdweights that reload the stationary already in the PE
    array (same weights AP, only non-loading Matmults in between). Waits on
    a removed load are pushed onto the next PE instruction; loads carrying
    semaphore updates are kept."""
    removed = 0
    for f in nc.m.functions:
        for blk in f.blocks:
            cur = None
            out = []
            pending_waits = []
            for inst in blk.instructions:
                if isinstance(inst, mybir.InstLdweights):
                    sig = _ldw_sig(inst)
                    si = inst.sync_info
                    has_upd = si is not None and len(si.on_update) > 0
                    if sig == cur and not has_upd:
                        if si is not None and len(si.on_wait) > 0:
                            pending_waits.extend(si.on_wait)
                        removed += 1
                        continue
                    cur = sig
                elif isinstance(inst, mybir.InstMatmult):
                    if inst.is_transpose or getattr(inst, 'ldweights', None) is not False:
                        cur = None
                elif type(inst).__name__ == 'InstMatmultMx':
                    cur = None
                if pending_waits and isinstance(
                        inst, (mybir.InstLdweights, mybir.InstMatmult)):
                    si = inst.sync_info
                    if si is None:
                        inst.sync_info = mybir.SyncInfo(
                            on_wait=list(pending_waits), on_update=[])
                    else:
                        si.on_wait = list(si.on_wait) + pending_waits
                    pending_waits = []
                out.append(inst)
            assert not pending_waits
            if len(out) != len(blk.instructions):
                blk.instructions[:] = out
    return removed


def _build(iters: int):
    nc = bacc.Bacc("TRN2", target_bir_lowering=False, debug=False)
    x_d = nc.dram_tensor("x", [SLAB_R, SLAB_C], BF16, kind="ExternalInput").ap()
    tri_d = nc.dram_tensor("tri", [128, 128], BF16, kind="ExternalInput").ap()
    m16_d = nc.dram_tensor("m16", [128, 128], BF16, kind="ExternalInput").ap()
    rmask_d = nc.dram_tensor("rmask", [NT, 128], F32, kind="ExternalInput").ap()
    out_d = nc.dram_tensor("ysum", [1, 1], F32, kind="ExternalOutput").ap()

    add = mybir.AluOpType.add

    with tile.TileContext(nc) as tc, ExitStack() as ctx:
        const_pool = ctx.enter_context(tc.tile_pool(name="const", bufs=1))
        ypool = ctx.enter_context(tc.tile_pool(name="y", bufs=1))
        bpool = ctx.enter_context(tc.tile_pool(name="b", bufs=1))

        tri_sb = const_pool.tile([128, 128], BF16, tag="tri")
        nc.sync.dma_start(tri_sb[:], tri_d[:])
        m16_sb = const_pool.tile([128, 128], BF16, tag="m16")
        nc.sync.dma_start(m16_sb[:], m16_d[:])
        rmask_sb = []
        for t in range(NT):
            rm = const_pool.tile([128, 1], F32, tag=f"rmask{t}", name=f"rmask{t}")
            nc.sync.dma_start(rm[:], rmask_d[t:t + 1, :])
            rmask_sb.append(rm)
        bias_sb = const_pool.tile([128, 1], F32, tag="biasc", name="biasc")
        nc.gpsimd.memset(bias_sb[:], -2460.0)

        y_sb = [ypool.tile([RT[t], SLAB_C], BF16, tag=f"y{t}", name=f"y{t}")
                for t in range(NT)]
        b_sb = [bpool.tile([RT[t], W], BF16, tag=f"b{t}", name=f"b{t}")
                for t in range(NT)]
        # Hy scratch for S chunks (b + center); S chunks are the trailing
        # S_CHUNKS[t]*512 columns of tile t
        SW = [S_CHUNKS[t] * MMW for t in range(NT)]      # S width per tile
        SC0 = [W - SW[t] for t in range(NT)]             # S col start
        hy_sb = {t: bpool.tile([RT[t], SW[t]], BF16, tag=f"hy{t}",
                               name=f"hy{t}")
                 for t in range(NT) if SW[t] > 0}

        # load (host already converted to bf16)
        for t in range(NT):
            nc.sync.dma_start(y_sb[t][:], x_d[OFF[t]:OFF[t] + RT[t], :])

        def emit_adds(t):
            r = RT[t]
            # b = l + r over all cols
            nc.vector.tensor_tensor(
                b_sb[t][:], y_sb[t][:, 0:W], y_sb[t][:, 2:W + 2], op=add)
            # Hy = b + center for the S range
            if SW[t] > 0:
                c0 = SC0[t]
                nc.vector.tensor_tensor(
                    hy_sb[t][:], b_sb[t][0:r, c0:W],
                    y_sb[t][:, 1 + c0:1 + W], op=add)

        def emit_seam(t):
            # refresh the 2*KSH-row overlap between tiles t and t+1 (each
            # tile's outer KSH rows go stale over KSH iterations)
            nc.sync.dma_start(y_sb[t][128 - KSH:128, :],
                              y_sb[t + 1][KSH:2 * KSH, :])
            nc.sync.dma_start(y_sb[t + 1][0:KSH, :],
                              y_sb[t][STRIDE:STRIDE + KSH, :])

        acc_list = []  # (tile, acc_tile) pairs written on the last iteration

        def mm(first, *args, **kw):
            if first:
                nc.tensor.matmul(*args, **kw)
            else:
                with _no_ldweights():
                    nc.tensor.matmul(*args, **kw)

        def emit_mms_thresholds(psum_pool, it, t, accum=False):
            r = RT[t]
            psums = [psum_pool.tile([r, PAIRW], F32, tag="ps",
                                    name=f"ps_{it}_{t}_{p}")
                     for p in range(NPAIR)]
            n_chunks = W // MMW
            a_end = n_chunks - S_CHUNKS[t]   # chunks [0, a_end) are 'A'

            def tri_group(p, first, a_start):
                # tri stream: A chunks read b, S chunks read hy
                for ch in range(p * MPP, (p + 1) * MPP):
                    h = ch - p * MPP
                    dst = psums[p][:, h * MMW:(h + 1) * MMW]
                    c0 = ch * MMW
                    if ch < a_end:
                        mm(first, dst, tri_sb[0:r, 0:r],
                           b_sb[t][0:r, c0:c0 + MMW],
                           start=a_start, stop=not a_start)
                    else:
                        mm(first, dst, tri_sb[0:r, 0:r],
                           hy_sb[t][0:r, c0 - SC0[t]:c0 - SC0[t] + MMW],
                           start=True, stop=True)
                    first = False

            def m16_group(p, first, a_start):
                for ch in range(p * MPP, min((p + 1) * MPP, a_end)):
                    h = ch - p * MPP
                    c0 = ch * MMW
                    mm(first, psums[p][:, h * MMW:(h + 1) * MMW],
                       m16_sb[0:r, 0:r],
                       y_sb[t][:, 1 + c0:1 + c0 + MMW],
                       start=a_start, stop=not a_start)
                    first = False

            # Per-pair stationary groups, ordered [p0: tri,m16][p1: m16,tri]
            # so pair-0's threshold can start one pair earlier (PSUM slot
            # pipelining across tiles) while group boundaries still merge
            # LDWEIGHTS: p0 ends m16 / p1 begins m16, and p1 ends tri /
            # next tile's p0 begins tri (dedup removes the reload).
            tri_group(0, True, True)
            m16_group(0, True, False)
            m16_group(1, True, True)
            tri_group(1, True, a_end <= MPP)

            def acc_for(kind):
                if not accum:
                    return None
                a = const_pool.tile([128, 1], F32, tag=f"acc{t}_{kind}",
                                    name=f"acc{t}_{kind}")
                acc_list.append((t, a))
                return a[0:r, 0:1]

            for p in range(NPAIR):
                lo, hi = p * MPP, (p + 1) * MPP       # chunk range of pair
                a_hi = min(hi, a_end)
                if a_hi > lo:                          # A sub-range: sigmoid
                    c0, c1 = lo * MMW, a_hi * MMW
                    nc.scalar.activation(
                        y_sb[t][:, 1 + c0:1 + c1],
                        psums[p][:, c0 - lo * MMW:c1 - lo * MMW],
                        mybir.ActivationFunctionType.Sigmoid,
                        bias=bias_sb[0:r, 0:1], scale=120.0,
                        accum_out=acc_for(f"{p}a"))
                if hi > max(lo, a_end):                # S sub-range: fused STT
                    c0, c1 = max(lo, a_end) * MMW, hi * MMW
                    ysl = y_sb[t][:, 1 + c0:1 + c1]
                    nc.vector.scalar_tensor_tensor(
                        ysl, psums[p][:, c0 - lo * MMW:c1 - lo * MMW],
                        4.5, ysl,
                        op0=mybir.AluOpType.is_gt,
                        op1=mybir.AluOpType.mult,
                        accum_out=acc_for(f"{p}s"))

        # Software-pipelined wavefront with seam shrinkage: tiles overlap by
        # 2*KSH rows, so seams need refreshing only every KSH-th iteration.
        # On non-refresh boundaries a tile's next-iteration adds depend only
        # on its own thresholds and are emitted right after it -- TensorE
        # rolls across the iteration boundary with no bubble. On refresh
        # boundaries, seams are refreshed as soon as both neighbor tiles are
        # thresholded.
        with tc.tile_pool(name="ps", bufs=2, space="PSUM") as psum_pool:
            for t in range(NT):
                emit_adds(t)
            for it in range(iters):
                last = it == iters - 1
                refresh = (it % KSH == KSH - 1) and not last
                for t in range(NT):
                    emit_mms_thresholds(psum_pool, it, t, accum=last)
                    if last:
                        continue
                    if refresh:
                        if t >= 1:
                            emit_seam(t - 1)
                        if t >= 2:
                            emit_adds(t - 2)
                    else:
                        emit_adds(t)
                if not last and refresh:
                    emit_adds(NT - 2)
                    emit_adds(NT - 1)

        # masked dot of the per-row accumulators from the last iteration's
        # thresholds: ysum = sum_t rmask[t] . (row sums of tile t)
        with tc.tile_pool(name="sps", bufs=1, space="PSUM") as spsum_pool:
            sps = spsum_pool.tile([1, 1], F32, tag="sum", name="sps")
            n_mm = len(acc_list)
            for k, (t, a) in enumerate(acc_list):
                nc.tensor.matmul(
                    sps[:], rmask_sb[t][0:RT[t], 0:1],
                    a[0:RT[t], 0:1],
                    start=(k == 0), stop=(k == n_mm - 1))
            ssb = const_pool.tile([1, 1], F32, tag="ssum", name="ssb")
            nc.vector.tensor_copy(ssb[:], sps[:])
            nc.sync.dma_start(out_d[:], ssb[:])

    _dedup_ldweights(nc)
    # After dedup, the "most recent ldweights" a matmul's extra waits would
    # be moved to can sit many matmuls earlier in the PE stream — waiting
    # there can deadlock against producers scheduled in between. Skip the
    # pass; generate_event_semaphores enforces the 1-wait constraint by
    # splitting waits into standalone event-sem instructions in place.
    nc.move_matmul_waits_to_ldweights = lambda: None
    nc.compile()
    return nc


def _consts():
    i = np.arange(128)
    tri = (np.abs(i[:, None] - i[None, :]) <= 1).astype(np.float32)
    m16 = tri + 16.0 * np.eye(128, dtype=np.float32)
    # valid-row masks for the final sum: slab rows [33, 545) are the owned
    # 512 rows; each row is summed from the tile where it is seam-valid
    # (interior partitions after the last iteration).
    rmask = np.zeros((NT, 128), np.float32)
    bounds = [(33, 124), (4, 124), (4, 124), (4, 124), (4, 65)]
    for t, (a, b) in enumerate(bounds):
        rmask[t, a:b] = 1.0
    assert sum(b - a for a, b in bounds) == OWN
    bf = ml_dtypes.bfloat16
    return tri.astype(bf), m16.astype(bf), rmask


def _slabs(x: np.ndarray):
    g = np.zeros((H + 2 * HALO + 2, SLAB_C), ml_dtypes.bfloat16)
    g[HALO + 1:HALO + 1 + H, 1:1 + W] = x  # 0/1 values: exact in bf16
    return [np.ascontiguousarray(g[c * OWN:c * OWN + SLAB_R])
            for c in range(NCORES)]


_CACHE = {}


def _get_nc(iters: int):
    if iters not in _CACHE:
        _CACHE[iters] = _build(iters)
    return _CACHE[iters]


def kernel(x: np.ndarray, convs) -> np.ndarray:
    iters = int(convs)
    x = np.asarray(x, np.float32)
    assert x.shape == (H, W)
    nc = _get_nc(iters)
    tri, m16, rmask = _consts()
    in_maps = [{"x": s, "tri": tri, "m16": m16, "rmask": rmask}
               for s in _slabs(x)]
    res = run_bass_kernel_spmd(nc, in_maps, core_ids=list(range(NCORES)))
    y_sum = sum(float(res.results[c]["ysum"][0, 0]) for c in range(NCORES))
    x_sum = float(x.astype(np.float64).sum())
    return np.float32(x_sum - y_sum)


if __name__ == "__main__":
    rng = np.random.default_rng(0)
    x = np.round(rng.random((H, W))).astype(np.float32)
    got = kernel(x, 32)
    from scipy import signal
    K = np.array([[1, 1, 1], [1, 0, 1], [1, 1, 1]], np.float32)
    y = x.copy()
    for _ in range(32):
        s = signal.convolve2d(y, K, mode='same')
        y = np.where(s > 3.0, y, 0).astype(np.float32)
    want = x.sum(dtype=np.float64) - y.sum(dtype=np.float64)
    print(f"got {got}, want {want}, rel {abs(got - want) / abs(want):.3e}")


# revision 11
# speedup vs baseline: 1.2481x; 1.0152x over previous
"""Trainium2 Bass kernel for the 32-iteration 3x3 survival automaton.

Problem: x is a 4096x4096 binary fp32 grid. 32 iterations of:
    keep cell iff its 8-neighbor live count > 3  (zero 'SAME' padding)
Output: scalar sum(x) - sum(y_final).

Strategy (8 NeuronCores, SPMD, zero inter-core communication):
  - Row-shard: core c owns rows [512c, 512c+512) and loads them plus a
    32-row halo per side; the halo is consumed one row per iteration, so
    after 32 iterations the owned rows are exact with no core-to-core
    traffic. One guard row/col of zeros emulates the 'SAME' zero padding
    (dead cells stay dead, so guards self-maintain).
  - Per-core slab: 578 rows x 4098 cols bf16, five 128-partition row tiles
    (stride 120, 8-row overlap; seam rows refreshed by DMAs every KSH
    iterations).
  - Work is balanced across ALL FOUR compute engines per iteration:
      TensorE: vertical 3-tap conv as tridiagonal matmuls. Columns are
        processed in 2048-wide pairs (4 PSUM banks). "A" pairs use two
        accumulated streams, Tri@B + (Tri+16I)@y, so one ScalarE sigmoid
        both thresholds and keeps y binary (s = N8 + 17y vs 20.5). "S"
        pairs use a single stream Tri@Hy (Hy = l+c+r) -- their threshold
        is a fused VectorE scalar_tensor_tensor (s > 4.5)*y, saving the
        second PE stream.
      ScalarE: sigmoid thresholds for A pairs (saturates to exact 1.0 /
        ~1e-26), batched 2048 wide, 8 pairs/iter.
      VectorE: B = l+r adds for tiles 0-1 (+ half of 4), Hy = B+y adds
        for S pairs, and the fused STT thresholds for S pairs.
      GpSimdE: B = l+r adds for tiles 2-3 (+ half of 4) -- otherwise idle.
  - TensorE stationaries: tri (all streams) and m16 = tri+16I (A second
    stream). Group emission order alternates per tile so dedup merges
    back-to-back reloads of the same stationary.
  - Final reduction: accum_out on the last iteration's thresholds gives
    per-partition row sums per pair; masked ones-vector matmuls reduce
    to one scalar per core. Host sums 8 partials, subtracts from sum(x).
"""

import sys

if '/opt/trn_rl_repo' not in sys.path:
    sys.path.insert(0, '/opt/trn_rl_repo')

from contextlib import ExitStack, contextmanager

import ml_dtypes
import numpy as np

import concourse.bass as bass
import concourse.tile as tile
from concourse import bacc, mybir
from concourse.bass_utils import run_bass_kernel_spmd

# ---------------------------------------------------------------- geometry
H = W = 4096
NCORES = 8
OWN = H // NCORES            # 512 rows owned per core
HALO = 32                    # rows of redundant compute per side
SLAB_R = OWN + 2 * HALO + 2  # 578 (incl. 1 guard row each side)
SLAB_C = W + 2               # 4098 (incl. 1 guard col each side)
NT = 5                       # SBUF row-tiles per slab
KSH = 7                      # seam shrink depth: refresh every KSH iters
STRIDE = 128 - 2 * KSH       # 120 (8-row overlap between tiles)
OFF = [t * STRIDE for t in range(NT)]              # 0,120,240,360,480
RT = [min(128, SLAB_R - o) for o in OFF]           # 128,128,128,128,98
PAIRW = 2048                 # threshold granularity: 4 PSUM banks
NPAIR = W // PAIRW           # 2 psum pairs per row-tile
MMW = 512                    # matmul output free size (1 PSUM bank)
MPP = PAIRW // MMW           # matmuls per pair per stationary (4)

# Per-tile count of trailing 512-col chunks (of 8) using the 'S' scheme:
# single-stream Tri@Hy + fused VectorE (s>4.5)*y threshold. The leading
# chunks use 'A': two-stream fold + ScalarE sigmoid. Tuned so PE, ACT and
# DVE land at the same per-iteration busy time (~16.7us).
S_CHUNKS = [0, 0, 0, 4, 3]

F32 = mybir.dt.float32
BF16 = mybir.dt.bfloat16


@contextmanager
def _no_ldweights():
    """Emit InstMatmult with ldweights=False: reuse the PE array's currently
    loaded stationary instead of reloading per matmul."""
    orig = mybir.InstMatmult

    def mk(*a, **kw):
        kw['ldweights'] = False
        return orig(*a, **kw)

    mybir.InstMatmult = mk
    try:
        yield
    finally:
        mybir.InstMatmult = orig


def _ldw_sig(inst):
    """Signature of the stationary operand an InstLdweights loads."""
    ap = inst.ins[0]
    return (getattr(ap, 'memref', None), getattr(ap, 'offset', None),
            str(getattr(ap, 'ap', None)), str(inst.tile_position),
            str(inst.tile_size), str(getattr(inst, 'perf_mode', None)),
            str(getattr(inst, 'is_transpose', None)))


def _dedup_ldweights(nc):
    """Remove InstLdweights that reload the stationary already in the PE
    array (same weights AP, only non-loading Matmults in between). Waits on
    a removed load are pushed onto the next PE instruction; loads carrying
    semaphore updates are kept."""
    removed = 0
    for f in nc.m.functions:
        for blk in f.blocks:
            cur = None
            out = []
            pending_waits = []
            for inst in blk.instructions:
                if isinstance(inst, mybir.InstLdweights):
                    sig = _ldw_sig(inst)
                    si = inst.sync_info
                    has_upd = si is not None and len(si.on_update) > 0
                    if sig == cur and not has_upd:
                        if si is not None and len(si.on_wait) > 0:
                            pending_waits.extend(si.on_wait)
                        removed += 1
                        continue
                    cur = sig
                elif isinstance(inst, mybir.InstMatmult):
                    if inst.is_transpose or getattr(inst, 'ldweights', None) is not False:
                        cur = None
                elif type(inst).__name__ == 'InstMatmultMx':
                    cur = None
                if pending_waits and isinstance(
                        inst, (mybir.InstLdweights, mybir.InstMatmult)):
                    si = inst.sync_info
                    if si is None:
                        inst.sync_info = mybir.SyncInfo(
                            on_wait=list(pending_waits), on_update=[])
                    else:
                        si.on_wait = list(si.on_wait) + pending_waits
                    pending_waits = []
                out.append(inst)
            assert not pending_waits
            if len(out) != len(blk.instructions):
                blk.instructions[:] = out
    return removed


def _build(iters: int):
    nc = bacc.Bacc("TRN2", target_bir_lowering=False, debug=False)
    x_d = nc.dram_tensor("x", [SLAB_R, SLAB_C], BF16, kind="ExternalInput").ap()
    tri_d = nc.dram_tensor("tri", [128, 128], BF16, kind="ExternalInput").ap()
    m16_d = nc.dram_tensor("m16", [128, 128], BF16, kind="ExternalInput").ap()
    rmask_d = nc.dram_tensor("rmask", [NT, 128], F32, kind="ExternalInput").ap()
    out_d = nc.dram_tensor("ysum", [1, 1], F32, kind="ExternalOutput").ap()

    add = mybir.AluOpType.add

    with tile.TileContext(nc) as tc, ExitStack() as ctx:
        const_pool = ctx.enter_context(tc.tile_pool(name="const", bufs=1))
        ypool = ctx.enter_context(tc.tile_pool(name="y", bufs=1))
        bpool = ctx.enter_context(tc.tile_pool(name="b", bufs=1))

        tri_sb = const_pool.tile([128, 128], BF16, tag="tri")
        nc.sync.dma_start(tri_sb[:], tri_d[:])
        m16_sb = const_pool.tile([128, 128], BF16, tag="m16")
        nc.sync.dma_start(m16_sb[:], m16_d[:])
        rmask_sb = []
        for t in range(NT):
            rm = const_pool.tile([128, 1], F32, tag=f"rmask{t}", name=f"rmask{t}")
            nc.sync.dma_start(rm[:], rmask_d[t:t + 1, :])
            rmask_sb.append(rm)
        bias_sb = const_pool.tile([128, 1], F32, tag="biasc", name="biasc")
        nc.gpsimd.memset(bias_sb[:], -2460.0)

        y_sb = [ypool.tile([RT[t], SLAB_C], BF16, tag=f"y{t}", name=f"y{t}")
                for t in range(NT)]
        b_sb = [bpool.tile([RT[t], W], BF16, tag=f"b{t}", name=f"b{t}")
                for t in range(NT)]
        # Hy scratch for S chunks (b + center); S chunks are the trailing
        # S_CHUNKS[t]*512 columns of tile t
        SW = [S_CHUNKS[t] * MMW for t in range(NT)]      # S width per tile
        SC0 = [W - SW[t] for t in range(NT)]             # S col start
        hy_sb = {t: bpool.tile([RT[t], SW[t]], BF16, tag=f"hy{t}",
                               name=f"hy{t}")
                 for t in range(NT) if SW[t] > 0}

        # load (host already converted to bf16)
        for t in range(NT):
            nc.sync.dma_start(y_sb[t][:], x_d[OFF[t]:OFF[t] + RT[t], :])

        def emit_adds(t):
            r = RT[t]
            # b = l + r over all cols
            nc.vector.tensor_tensor(
                b_sb[t][:], y_sb[t][:, 0:W], y_sb[t][:, 2:W + 2], op=add)
            # Hy = b + center for the S range
            if SW[t] > 0:
                c0 = SC0[t]
                nc.vector.tensor_tensor(
                    hy_sb[t][:], b_sb[t][0:r, c0:W],
                    y_sb[t][:, 1 + c0:1 + W], op=add)

        def emit_seam(t):
            # refresh the 2*KSH-row overlap between tiles t and t+1 (each
            # tile's outer KSH rows go stale over KSH iterations)
            nc.sync.dma_start(y_sb[t][128 - KSH:128, :],
                              y_sb[t + 1][KSH:2 * KSH, :])
            nc.sync.dma_start(y_sb[t + 1][0:KSH, :],
                              y_sb[t][STRIDE:STRIDE + KSH, :])

        acc_list = []  # (tile, acc_tile) pairs written on the last iteration

        def mm(first, *args, **kw):
            if first:
                nc.tensor.matmul(*args, **kw)
            else:
                with _no_ldweights():
                    nc.tensor.matmul(*args, **kw)

        def emit_mms_thresholds(psum_pool, it, t, accum=False):
            r = RT[t]
            psums = [psum_pool.tile([r, PAIRW], F32, tag="ps",
                                    name=f"ps_{it}_{t}_{p}")
                     for p in range(NPAIR)]
            n_chunks = W // MMW
            a_end = n_chunks - S_CHUNKS[t]   # chunks [0, a_end) are 'A'

            def tri_group(p, first, a_start):
                # tri stream: A chunks read b, S chunks read hy
                for ch in range(p * MPP, (p + 1) * MPP):
                    h = ch - p * MPP
                    dst = psums[p][:, h * MMW:(h + 1) * MMW]
                    c0 = ch * MMW
                    if ch < a_end:
                        mm(first, dst, tri_sb[0:r, 0:r],
                           b_sb[t][0:r, c0:c0 + MMW],
                           start=a_start, stop=not a_start)
                    else:
                        mm(first, dst, tri_sb[0:r, 0:r],
                           hy_sb[t][0:r, c0 - SC0[t]:c0 - SC0[t] + MMW],
                           start=True, stop=True)
                    first = False

            def m16_group(p, first, a_start):
                for ch in range(p * MPP, min((p + 1) * MPP, a_end)):
                    h = ch - p * MPP
                    c0 = ch * MMW
                    mm(first, psums[p][:, h * MMW:(h + 1) * MMW],
                       m16_sb[0:r, 0:r],
                       y_sb[t][:, 1 + c0:1 + c0 + MMW],
                       start=a_start, stop=not a_start)
                    first = False

            # Per-pair stationary groups, ordered [p0: tri,m16][p1: m16,tri]
            # so pair-0's threshold can start one pair earlier (PSUM slot
            # pipelining across tiles) while group boundaries still merge
            # LDWEIGHTS: p0 ends m16 / p1 begins m16, and p1 ends tri /
            # next tile's p0 begins tri (dedup removes the reload).
            tri_group(0, True, True)
            m16_group(0, True, False)
            m16_group(1, True, True)
            tri_group(1, True, a_end <= MPP)

            def acc_for(kind):
                if not accum:
                    return None
                a = const_pool.tile([128, 1], F32, tag=f"acc{t}_{kind}",
                                    name=f"acc{t}_{kind}")
                acc_list.append((t, a))
                return a[0:r, 0:1]

            for p in range(NPAIR):
                lo, hi = p * MPP, (p + 1) * MPP       # chunk range of pair
                a_hi = min(hi, a_end)
                if a_hi > lo:                          # A sub-range: sigmoid
                    c0, c1 = lo * MMW, a_hi * MMW
                    nc.scalar.activation(
                        y_sb[t][:, 1 + c0:1 + c1],
                        psums[p][:, c0 - lo * MMW:c1 - lo * MMW],
                        mybir.ActivationFunctionType.Sigmoid,
                        bias=bias_sb[0:r, 0:1], scale=120.0,
                        accum_out=acc_for(f"{p}a"))
                if hi > max(lo, a_end):                # S sub-range: fused STT
                    c0, c1 = max(lo, a_end) * MMW, hi * MMW
                    ysl = y_sb[t][:, 1 + c0:1 + c1]
                    nc.vector.scalar_tensor_tensor(
                        ysl, psums[p][:, c0 - lo * MMW:c1 - lo * MMW],
                        4.5, ysl,
                        op0=mybir.AluOpType.is_gt,
                        op1=mybir.AluOpType.mult,
                        accum_out=acc_for(f"{p}s"))

        # Software-pipelined wavefront with seam shrinkage: tiles overlap by
        # 2*KSH rows, so seams need refreshing only every KSH-th iteration.
        # On non-refresh boundaries a tile's next-iteration adds depend only
        # on its own thresholds and are emitted right after it -- TensorE
        # rolls across the iteration boundary with no bubble. On refresh
        # boundaries, seams are refreshed as soon as both neighbor tiles are
        # thresholded.
        with tc.tile_pool(name="ps", bufs=2, space="PSUM") as psum_pool:
            for t in range(NT):
                emit_adds(t)
            for it in range(iters):
                last = it == iters - 1
                refresh = (it % KSH == KSH - 1) and not last
                for t in range(NT):
                    emit_mms_thresholds(psum_pool, it, t, accum=last)
                    if last:
                        continue
                    if refresh:
                        if t >= 1:
                            emit_seam(t - 1)
                        if t >= 2:
                            emit_adds(t - 2)
                    else:
                        emit_adds(t)
                if not last and refresh:
                    emit_adds(NT - 2)
                    emit_adds(NT - 1)

        # masked dot of the per-row accumulators from the last iteration's
        # thresholds: ysum = sum_t rmask[t] . (row sums of tile t)
        with tc.tile_pool(name="sps", bufs=1, space="PSUM") as spsum_pool:
            sps = spsum_pool.tile([1, 1], F32, tag="sum", name="sps")
            n_mm = len(acc_list)
            for k, (t, a) in enumerate(acc_list):
                nc.tensor.matmul(
                    sps[:], rmask_sb[t][0:RT[t], 0:1],
                    a[0:RT[t], 0:1],
                    start=(k == 0), stop=(k == n_mm - 1))
            ssb = const_pool.tile([1, 1], F32, tag="ssum", name="ssb")
            nc.vector.tensor_copy(ssb[:], sps[:])
            nc.sync.dma_start(out_d[:], ssb[:])

    _dedup_ldweights(nc)
    # After dedup, the "most recent ldweights" a matmul's extra waits would
    # be moved to can sit many matmuls earlier in the PE stream — waiting
    # there can deadlock against producers scheduled in between. Skip the
    # pass; generate_event_semaphores enforces the 1-wait constraint by
    # splitting waits into standalone event-sem instructions in place.
    nc.move_matmul_waits_to_ldweights = lambda: None
    nc.compile()
    return nc


def _consts():
    i = np.arange(128)
    tri = (np.abs(i[:, None] - i[None, :]) <= 1).astype(np.float32)
    m16 = tri + 16.0 * np.eye(128, dtype=np.float32)
    # valid-row masks for the final sum: slab rows [33, 545) are the owned
    # 512 rows; each row is summed from the tile where it is seam-valid
    # (interior partitions after the last iteration).
    rmask = np.zeros((NT, 128), np.float32)
    # interior partitions [KSH, 128-KSH) are seam-valid; tile 0 has no
    # upper seam (slab edge) and tile 4 no lower seam
    bounds = [(33, 121), (7, 121), (7, 121), (7, 121), (7, 89)]
    for t, (a, b) in enumerate(bounds):
        rmask[t, a:b] = 1.0
    assert sum(b - a for a, b in bounds) == OWN
    bf = ml_dtypes.bfloat16
    return tri.astype(bf), m16.astype(bf), rmask


def _slabs(x: np.ndarray):
    g = np.zeros((H + 2 * HALO + 2, SLAB_C), ml_dtypes.bfloat16)
    g[HALO + 1:HALO + 1 + H, 1:1 + W] = x  # 0/1 values: exact in bf16
    return [np.ascontiguousarray(g[c * OWN:c * OWN + SLAB_R])
            for c in range(NCORES)]


_CACHE = {}


def _get_nc(iters: int):
    if iters not in _CACHE:
        _CACHE[iters] = _build(iters)
    return _CACHE[iters]


def kernel(x: np.ndarray, convs) -> np.ndarray:
    iters = int(convs)
    x = np.asarray(x, np.float32)
    assert x.shape == (H, W)
    nc = _get_nc(iters)
    tri, m16, rmask = _consts()
    in_maps = [{"x": s, "tri": tri, "m16": m16, "rmask": rmask}
               for s in _slabs(x)]
    res = run_bass_kernel_spmd(nc, in_maps, core_ids=list(range(NCORES)))
    y_sum = sum(float(res.results[c]["ysum"][0, 0]) for c in range(NCORES))
    x_sum = float(x.astype(np.float64).sum())
    return np.float32(x_sum - y_sum)


if __name__ == "__main__":
    rng = np.random.default_rng(0)
    x = np.round(rng.random((H, W))).astype(np.float32)
    got = kernel(x, 32)
    from scipy import signal
    K = np.array([[1, 1, 1], [1, 0, 1], [1, 1, 1]], np.float32)
    y = x.copy()
    for _ in range(32):
        s = signal.convolve2d(y, K, mode='same')
        y = np.where(s > 3.0, y, 0).astype(np.float32)
    want = x.sum(dtype=np.float64) - y.sum(dtype=np.float64)
    print(f"got {got}, want {want}, rel {abs(got - want) / abs(want):.3e}")
